# revision 11
# baseline (speedup 1.0000x reference)
"""Cross-attention Trainium2 kernel (Bass/Tile), sharded over 8 NeuronCores.

Problem: B=2, Sq=Sk=2048, H=16, D=64, fp32, with a boolean key-padding mask
(True = keep).  softmax(q @ k^T / sqrt(D) + mask_bias) @ v per (batch, head).

Sharding: the 32 (b, h) pairs are split 4-per-core (cores 0-3 -> b=0,
cores 4-7 -> b=1).  Masked-out keys are compacted away on the host (the
reference's additive -10000 bias makes exp() underflow to exactly 0 in fp32,
so dropping those keys is exact); the kept keys are padded with zero rows up
to a multiple of 128.  Padding rows contribute exp(0)=1 "probabilities", but
their V rows AND their entry in the appended ones-column are 0, so they add
exactly nothing to either numerator or denominator.

Device math per (core, head h, 1024-query chunk c), T = Ske/128 key tiles:
  S^T[t]   = K^T[t].T @ Q^T            (PE; [128 keys, 1024 queries], c=D=64)
  P[t]     = exp(S^T[t])               (split across TWO engines:
               ~60%% of tiles: ACT exp PSUM->SBUF(bf16)
               ~40%% of tiles: DVE copy PSUM->SBUF, then GPSIMD pow(e, s)
               -- the 1/sqrt(D) scale is folded into K on the host)
  O[j]    += P[t][:,j*128:...].T @ V1[t]   (PE; V1 = [V | ones] bf16,
                                            [128 queries, 65] psum acc)
  out      = O[:, :64] * 1/O[:, 64]    (DVE reciprocal + broadcast mult)

Keeping queries on the PSUM partition dim for the PV accumulation (free dim
65 instead of 512) halves PE time vs streaming P columns, and removes the
transpose epilogue entirely.  P/V in bf16 keep the PV matmuls at 1 cyc/row.
No row-max subtraction is needed: scores are ~N(0,1), exp() cannot overflow
fp32 and matches softmax-with-max to ~1e-6.
"""
import math
import numpy as np
from contextlib import ExitStack

import ml_dtypes

import concourse.bass as bass
import concourse.tile as tile
from concourse import bacc, mybir
from concourse.bass_utils import run_bass_kernel_spmd

f32 = mybir.dt.float32
f32r = mybir.dt.float32r  # tf32-like matmul dtype: 1 cyc/row vs 4 for fp32
bf16 = mybir.dt.bfloat16

B, Sq, Sk, H, D = 2, 2048, 2048, 16, 64
N_CORES = 8
CPB = N_CORES // B          # cores per batch item (4)
HPC = H // CPB              # heads per core (4)
NCHUNK = Sq // 1024         # query chunks of 1024 per head (2)

_CACHE: dict[int, "bacc.Bacc"] = {}

# greedy channel-balancer weights: engine-busy ns per [128,1024] tile
_ACT_COST = 1038.0   # 1024/1.2GHz + psum/sbuf access
_POOL_COST = 1517.0  # q7 launch + 1024/(1.2GHz*0.6)


def _build_program(T: int) -> "bacc.Bacc":
    """Build + compile the per-core Bass program for Ske = T*128 kept keys."""
    Ske = T * 128
    nc = bacc.Bacc("TRN2", target_bir_lowering=False, debug=False)

    qT = nc.dram_tensor("qT", [D, HPC, Sq], f32r, kind="ExternalInput").ap()
    kT = nc.dram_tensor("kT", [D, HPC, Ske], f32r, kind="ExternalInput").ap()
    v1 = nc.dram_tensor("v1", [128, HPC, T, D + 1], bf16, kind="ExternalInput").ap()
    o = nc.dram_tensor("o", [HPC, NCHUNK, 128, 8, D], f32, kind="ExternalOutput").ap()

    Exp = mybir.ActivationFunctionType.Exp
    E_CONST = float(math.e)

    with tile.TileContext(nc) as tc, ExitStack() as ctx:
        const = ctx.enter_context(tc.tile_pool(name="const", bufs=1))
        ring = ctx.enter_context(tc.tile_pool(name="ring", bufs=3, space="PSUM"))
        accp = ctx.enter_context(tc.tile_pool(name="accp", bufs=2, space="PSUM"))
        ppool = ctx.enter_context(tc.tile_pool(name="ppool", bufs=18))
        scp = ctx.enter_context(tc.tile_pool(name="scp", bufs=5))
        osbp = ctx.enter_context(tc.tile_pool(name="osbp", bufs=3))
        rp = ctx.enter_context(tc.tile_pool(name="rp", bufs=3))

        # warm the ACT exp table before the main loop
        warm = const.tile([128, 1], f32, tag="warm", name="warm")
        nc.gpsimd.memset(warm[:], 0.0)
        nc.scalar.activation(warm[:], warm[:], Exp, scale=1.0)

        # e-filled tile: base operand for the GPSIMD pow(e, s) exp channel
        e_sb = const.tile([128, 1024], f32, tag="e", name="e_sb")
        nc.vector.memset(e_sb[:], E_CONST)

        q_sb = const.tile([D, HPC, Sq], f32r, tag="q", name="q_sb")
        k_sb = const.tile([D, HPC, Ske], f32r, tag="k", name="k_sb")
        v_sb = const.tile([128, HPC, T, D + 1], bf16, tag="v", name="v_sb")
        # stage the critical first tiles first; keep ALL loads off the scalar
        # queue (a DMA issue occupies the ACT sequencer for >1us and delays
        # the first exp by several us)
        nc.sync.dma_start(k_sb[:, 0, :128], kT[:, 0, :128])
        nc.sync.dma_start(q_sb[:, 0, :512], qT[:, 0, :512])
        nc.sync.dma_start(q_sb[:, 0, 512:1024], qT[:, 0, 512:1024])
        if Ske > 128:
            nc.sync.dma_start(k_sb[:, 0, 128:], kT[:, 0, 128:])
        nc.sync.dma_start(q_sb[:, 0, 1024:], qT[:, 0, 1024:])
        for h in range(1, HPC):
            nc.sync.dma_start(k_sb[:, h], kT[:, h])
            nc.sync.dma_start(q_sb[:, h], qT[:, h])
        nc.gpsimd.dma_start(v_sb[:], v1[:])

        vheads = [(h, c) for h in range(HPC) for c in range(NCHUNK)]
        vacc = {}            # (h, c) -> (acc0, acc1) psum tiles
        act_load = 0.0       # greedy channel balancer state
        pool_load = 0.0

        def emit_pv(h, c, t, p):
            if t == 0:
                a0 = accp.tile([128, 4, D + 1], f32, tag="acc", name="a0")
                a1 = accp.tile([128, 4, D + 1], f32, tag="acc", name="a1")
                vacc[(h, c)] = (a0, a1)
            a0, a1 = vacc[(h, c)]
            # PSUM start/stop semantics are per 2KB zero-region (bank): the
            # FIRST matmul into a bank carries start=True (clears the whole
            # bank's has_written bits); the other 3 groups' first writes then
            # overwrite their own pending-zero bytes.  One stop on the last
            # matmul into the bank.
            for j in range(8):
                acc = a0 if j < 4 else a1
                nc.tensor.matmul(
                    acc[:, j % 4, :], p[:, j * 128:(j + 1) * 128],
                    v_sb[:, h, t],
                    start=(t == 0 and j % 4 == 0),
                    stop=(t == T - 1 and j % 4 == 3),
                )
            if t == T - 1:
                emit_epilogue(h, c)

        def emit_epilogue(h, c):
            a0, a1 = vacc.pop((h, c))
            r = rp.tile([128, 8], f32, tag="r", name="r")
            nc.vector.reciprocal(r[:, 0:4], a0[:, :, D])
            nc.vector.reciprocal(r[:, 4:8], a1[:, :, D])
            ot = osbp.tile([128, 8, D], f32, tag="ot", name="ot")
            nc.vector.tensor_tensor(
                ot[:, 0:4, :], a0[:, :, :D],
                r[:, 0:4, None].to_broadcast((128, 4, D)), mybir.AluOpType.mult,
            )
            nc.vector.tensor_tensor(
                ot[:, 4:8, :], a1[:, :, :D],
                r[:, 4:8, None].to_broadcast((128, 4, D)), mybir.AluOpType.mult,
            )
            nc.sync.dma_start(o[h, c], ot[:])

        # vhead-phase software pipeline: while vhead v's scores/exps stream
        # through the ring, run the PV bursts of vhead v-1 (whose P tiles are
        # all ready, so they never stall the in-order PE queue).  PV bursts
        # are emitted BEFORE each score matmul so a ring-slot wait cannot
        # block ready PV work behind it.
        prev = None          # (h, c, [p tiles]) of the previous vhead
        LAG = 4              # last vhead: own-PV lag (units); A-chain P is
                             # ready ~1.3us after its scores, G-chain ~3us
        for vi, (h, c) in enumerate(vheads):
            last = vi == len(vheads) - 1
            ptiles = []
            for t in range(T):
                if prev is not None:
                    emit_pv(prev[0], prev[1], t, prev[2][t])
                if last and t >= LAG:
                    emit_pv(h, c, t - LAG, ptiles[t - LAG])
                ps = ring.tile([128, 1024], f32, tag="s", name="ps")
                for jj in range(2):
                    q0 = c * 1024 + jj * 512
                    nc.tensor.matmul(
                        ps[:, jj * 512:(jj + 1) * 512],
                        k_sb[:, h, t * 128:(t + 1) * 128],
                        q_sb[:, h, q0:q0 + 512],
                        start=True, stop=True,
                    )
                p = ppool.tile([128, 1024], bf16, tag="p", name="p")
                # the tail units of the last vhead go to ACT: its exp->PV
                # chain is ~2x shorter than the copy+pow chain, so the
                # post-loop PV bursts aren't left waiting on GPSIMD
                use_act = (last and t >= LAG) or (
                    act_load + _ACT_COST <= pool_load + _POOL_COST)
                if use_act:
                    act_load += _ACT_COST
                    nc.scalar.activation(p[:], ps[:], Exp, scale=1.0)
                else:
                    pool_load += _POOL_COST
                    s2 = scp.tile([128, 1024], f32, tag="s2", name="s2")
                    nc.vector.tensor_copy(out=s2[:], in_=ps[:])
                    nc.gpsimd.tensor_tensor(p[:], e_sb[:], s2[:],
                                            mybir.AluOpType.pow)
                ptiles.append(p)
            prev = (h, c, ptiles)
        for t in range(T - LAG, T):
            emit_pv(prev[0], prev[1], t, prev[2][t])

    nc.compile()
    return nc


def kernel(q, kv, key_padding_mask):
    q = np.ascontiguousarray(np.asarray(q, dtype=np.float32))
    kv = np.asarray(kv, dtype=np.float32)
    mask = np.asarray(key_padding_mask).astype(bool)
    k = kv[:, :, 0]  # (B, Sk, H, D)
    v = kv[:, :, 1]

    # Host-side compaction of masked-out keys (exact: exp(-10000) == 0 in
    # fp32).  If every key of a batch item is masked, the -10000 bias is a
    # constant and softmax ignores it -> fall back to keeping all keys.
    idxs = []
    for b in range(B):
        ix = np.nonzero(mask[b])[0]
        if len(ix) == 0:
            ix = np.arange(Sk)
        idxs.append(ix)
    T = int(np.ceil(max(len(ix) for ix in idxs) / 128))
    Ske = T * 128

    in_maps = []
    for c in range(N_CORES):
        b = c // CPB
        h0 = (c % CPB) * HPC
        ix = idxs[b]
        cnt = len(ix)

        qT = np.ascontiguousarray(q[b, :, h0:h0 + HPC, :].transpose(2, 1, 0))
        kT = np.zeros((D, HPC, Ske), np.float32)
        # fold the 1/sqrt(D) softmax scale into K (exact: 0.125 = 2^-3)
        kT[:, :, :cnt] = (k[b][ix][:, h0:h0 + HPC, :] * 0.125).transpose(2, 1, 0)
        v1_full = np.zeros((HPC, Ske, D + 1), np.float32)
        v1_full[:, :cnt, :D] = v[b][ix][:, h0:h0 + HPC, :].transpose(1, 0, 2)
        v1_full[:, :cnt, D] = 1.0
        v1 = np.ascontiguousarray(
            v1_full.reshape(HPC, T, 128, D + 1).transpose(2, 0, 1, 3)
        ).astype(ml_dtypes.bfloat16)
        in_maps.append({"qT": qT, "kT": kT, "v1": v1})

    if T not in _CACHE:
        _CACHE[T] = _build_program(T)
    nc = _CACHE[T]

    res = run_bass_kernel_spmd(nc, in_maps, core_ids=list(range(N_CORES)))

    out = np.zeros((B, Sq, H, D), np.float32)
    for c in range(N_CORES):
        b = c // CPB
        h0 = (c % CPB) * HPC
        oc = res.results[c]["o"]  # (HPC, NCHUNK, 128, 8, D)
        for i in range(HPC):
            # row order within a chunk is (i_sub, p): q = c*1024 + i_sub*128 + p
            out[b, :, h0 + i, :] = oc[i].transpose(0, 2, 1, 3).reshape(Sq, D)
    return out


# revision 12
# speedup vs baseline: 1.0151x; 1.0151x over previous
"""Cross-attention Trainium2 kernel (Bass/Tile), sharded over 8 NeuronCores.

Problem: B=2, Sq=Sk=2048, H=16, D=64, fp32, with a boolean key-padding mask
(True = keep).  softmax(q @ k^T / sqrt(D) + mask_bias) @ v per (batch, head).

Sharding: the 32 (b, h) pairs are split 4-per-core (cores 0-3 -> b=0,
cores 4-7 -> b=1).  Masked-out keys are compacted away on the host (the
reference's additive -10000 bias makes exp() underflow to exactly 0 in fp32,
so dropping those keys is exact); the kept keys are padded with zero rows up
to a multiple of 128.  Padding rows contribute exp(0)=1 "probabilities", but
their V rows AND their entry in the appended ones-column are 0, so they add
exactly nothing to either numerator or denominator.

Device math per (core, head h, 1024-query chunk c), T = Ske/128 key tiles:
  S^T[t]   = K^T[t].T @ Q^T            (PE; [128 keys, 1024 queries], c=D=64)
  P[t]     = exp(S^T[t])               (split across TWO engines:
               ~60%% of tiles: ACT exp PSUM->SBUF(bf16)
               ~40%% of tiles: DVE copy PSUM->SBUF, then GPSIMD pow(e, s)
               -- the 1/sqrt(D) scale is folded into K on the host)
  O[j]    += P[t][:,j*128:...].T @ V1[t]   (PE; V1 = [V | ones] bf16,
                                            [128 queries, 65] psum acc)
  out      = O[:, :64] * 1/O[:, 64]    (DVE reciprocal + broadcast mult)

Keeping queries on the PSUM partition dim for the PV accumulation (free dim
65 instead of 512) halves PE time vs streaming P columns, and removes the
transpose epilogue entirely.  P/V in bf16 keep the PV matmuls at 1 cyc/row.
No row-max subtraction is needed: scores are ~N(0,1), exp() cannot overflow
fp32 and matches softmax-with-max to ~1e-6.
"""
import math
import numpy as np
from contextlib import ExitStack

import ml_dtypes

import concourse.bass as bass
import concourse.tile as tile
from concourse import bacc, mybir
from concourse.bass_utils import run_bass_kernel_spmd

f32 = mybir.dt.float32
f32r = mybir.dt.float32r  # tf32-like matmul dtype: 1 cyc/row vs 4 for fp32
bf16 = mybir.dt.bfloat16

B, Sq, Sk, H, D = 2, 2048, 2048, 16, 64
N_CORES = 8
CPB = N_CORES // B          # cores per batch item (4)
HPC = H // CPB              # heads per core (4)
NCHUNK = Sq // 1024         # query chunks of 1024 per head (2)

_CACHE: dict[int, "bacc.Bacc"] = {}

# greedy channel-balancer weights: engine-busy ns per [128,1024] tile
_ACT_COST = 1038.0   # 1024/1.2GHz + psum/sbuf access
_POOL_COST = 1517.0  # q7 launch + 1024/(1.2GHz*0.6)


def _build_program(T: int) -> "bacc.Bacc":
    """Build + compile the per-core Bass program for Ske = T*128 kept keys."""
    Ske = T * 128
    nc = bacc.Bacc("TRN2", target_bir_lowering=False, debug=False)

    qT = nc.dram_tensor("qT", [D, HPC, Sq], f32r, kind="ExternalInput").ap()
    kT = nc.dram_tensor("kT", [D, HPC, Ske], f32r, kind="ExternalInput").ap()
    v1 = nc.dram_tensor("v1", [128, HPC, T, D + 1], bf16, kind="ExternalInput").ap()
    o = nc.dram_tensor("o", [HPC, NCHUNK, 128, 8, D], f32, kind="ExternalOutput").ap()

    Exp = mybir.ActivationFunctionType.Exp
    E_CONST = float(math.e)

    with tile.TileContext(nc) as tc, ExitStack() as ctx:
        const = ctx.enter_context(tc.tile_pool(name="const", bufs=1))
        ring = ctx.enter_context(tc.tile_pool(name="ring", bufs=3, space="PSUM"))
        accp = ctx.enter_context(tc.tile_pool(name="accp", bufs=2, space="PSUM"))
        ppool = ctx.enter_context(tc.tile_pool(name="ppool", bufs=18))
        scp = ctx.enter_context(tc.tile_pool(name="scp", bufs=5))
        osbp = ctx.enter_context(tc.tile_pool(name="osbp", bufs=3))
        rp = ctx.enter_context(tc.tile_pool(name="rp", bufs=3))

        # warm the ACT exp table before the main loop
        warm = const.tile([128, 1], f32, tag="warm", name="warm")
        nc.gpsimd.memset(warm[:], 0.0)
        nc.scalar.activation(warm[:], warm[:], Exp, scale=1.0)

        # e-filled tile: base operand for the GPSIMD pow(e, s) exp channel
        e_sb = const.tile([128, 1024], f32, tag="e", name="e_sb")
        nc.vector.memset(e_sb[:], E_CONST)

        q_sb = const.tile([D, HPC, Sq], f32r, tag="q", name="q_sb")
        k_sb = const.tile([D, HPC, Ske], f32r, tag="k", name="k_sb")
        v_sb = const.tile([128, HPC, T, D + 1], bf16, tag="v", name="v_sb")
        # stage the critical first tiles first; keep ALL loads off the scalar
        # queue (a DMA issue occupies the ACT sequencer for >1us and delays
        # the first exp by several us)
        nc.sync.dma_start(k_sb[:, 0, :128], kT[:, 0, :128])
        nc.scalar.dma_start(q_sb[:, 0, :512], qT[:, 0, :512])
        nc.sync.dma_start(q_sb[:, 0, 512:1024], qT[:, 0, 512:1024])
        if Ske > 128:
            nc.sync.dma_start(k_sb[:, 0, 128:], kT[:, 0, 128:])
        nc.scalar.dma_start(q_sb[:, 0, 1024:], qT[:, 0, 1024:])
        for h in range(1, HPC):
            nc.sync.dma_start(k_sb[:, h], kT[:, h])
            nc.scalar.dma_start(q_sb[:, h], qT[:, h])
        nc.gpsimd.dma_start(v_sb[:], v1[:])

        vheads = [(h, c) for h in range(HPC) for c in range(NCHUNK)]
        vacc = {}            # (h, c) -> (acc0, acc1) psum tiles
        act_load = 0.0       # greedy channel balancer state
        pool_load = 0.0

        def emit_pv(h, c, t, p):
            if t == 0:
                a0 = accp.tile([128, 4, D + 1], f32, tag="acc", name="a0")
                a1 = accp.tile([128, 4, D + 1], f32, tag="acc", name="a1")
                vacc[(h, c)] = (a0, a1)
            a0, a1 = vacc[(h, c)]
            # PSUM start/stop semantics are per 2KB zero-region (bank): the
            # FIRST matmul into a bank carries start=True (clears the whole
            # bank's has_written bits); the other 3 groups' first writes then
            # overwrite their own pending-zero bytes.  One stop on the last
            # matmul into the bank.
            for j in range(8):
                acc = a0 if j < 4 else a1
                nc.tensor.matmul(
                    acc[:, j % 4, :], p[:, j * 128:(j + 1) * 128],
                    v_sb[:, h, t],
                    start=(t == 0 and j % 4 == 0),
                    stop=(t == T - 1 and j % 4 == 3),
                )
            if t == T - 1:
                emit_epilogue(h, c)

        def emit_epilogue(h, c):
            a0, a1 = vacc.pop((h, c))
            r = rp.tile([128, 8], f32, tag="r", name="r")
            nc.vector.reciprocal(r[:, 0:4], a0[:, :, D])
            nc.vector.reciprocal(r[:, 4:8], a1[:, :, D])
            ot = osbp.tile([128, 8, D], f32, tag="ot", name="ot")
            nc.vector.tensor_tensor(
                ot[:, 0:4, :], a0[:, :, :D],
                r[:, 0:4, None].to_broadcast((128, 4, D)), mybir.AluOpType.mult,
            )
            nc.vector.tensor_tensor(
                ot[:, 4:8, :], a1[:, :, :D],
                r[:, 4:8, None].to_broadcast((128, 4, D)), mybir.AluOpType.mult,
            )
            nc.sync.dma_start(o[h, c], ot[:])

        # vhead-phase software pipeline: while vhead v's scores/exps stream
        # through the ring, run the PV bursts of vhead v-1 (whose P tiles are
        # all ready, so they never stall the in-order PE queue).  PV bursts
        # are emitted BEFORE each score matmul so a ring-slot wait cannot
        # block ready PV work behind it.
        prev = None          # (h, c, [p tiles]) of the previous vhead
        LAG = 4              # last vhead: own-PV lag (units); A-chain P is
                             # ready ~1.3us after its scores, G-chain ~3us
        for vi, (h, c) in enumerate(vheads):
            last = vi == len(vheads) - 1
            ptiles = []
            for t in range(T):
                if prev is not None:
                    emit_pv(prev[0], prev[1], t, prev[2][t])
                if last and t >= LAG:
                    emit_pv(h, c, t - LAG, ptiles[t - LAG])
                ps = ring.tile([128, 1024], f32, tag="s", name="ps")
                for jj in range(2):
                    q0 = c * 1024 + jj * 512
                    nc.tensor.matmul(
                        ps[:, jj * 512:(jj + 1) * 512],
                        k_sb[:, h, t * 128:(t + 1) * 128],
                        q_sb[:, h, q0:q0 + 512],
                        start=True, stop=True,
                    )
                p = ppool.tile([128, 1024], bf16, tag="p", name="p")
                # the tail units of the last vhead go to ACT: its exp->PV
                # chain is ~2x shorter than the copy+pow chain, so the
                # post-loop PV bursts aren't left waiting on GPSIMD
                use_act = (last and t >= LAG) or (
                    act_load + _ACT_COST <= pool_load + _POOL_COST)
                if use_act:
                    act_load += _ACT_COST
                    nc.scalar.activation(p[:], ps[:], Exp, scale=1.0)
                else:
                    pool_load += _POOL_COST
                    s2 = scp.tile([128, 1024], f32, tag="s2", name="s2")
                    nc.vector.tensor_copy(out=s2[:], in_=ps[:])
                    nc.gpsimd.tensor_tensor(p[:], e_sb[:], s2[:],
                                            mybir.AluOpType.pow)
                ptiles.append(p)
            prev = (h, c, ptiles)
        for t in range(T - LAG, T):
            emit_pv(prev[0], prev[1], t, prev[2][t])

    nc.compile()
    return nc


def kernel(q, kv, key_padding_mask):
    q = np.ascontiguousarray(np.asarray(q, dtype=np.float32))
    kv = np.asarray(kv, dtype=np.float32)
    mask = np.asarray(key_padding_mask).astype(bool)
    k = kv[:, :, 0]  # (B, Sk, H, D)
    v = kv[:, :, 1]

    # Host-side compaction of masked-out keys (exact: exp(-10000) == 0 in
    # fp32).  If every key of a batch item is masked, the -10000 bias is a
    # constant and softmax ignores it -> fall back to keeping all keys.
    idxs = []
    for b in range(B):
        ix = np.nonzero(mask[b])[0]
        if len(ix) == 0:
            ix = np.arange(Sk)
        idxs.append(ix)
    T = int(np.ceil(max(len(ix) for ix in idxs) / 128))
    Ske = T * 128

    in_maps = []
    for c in range(N_CORES):
        b = c // CPB
        h0 = (c % CPB) * HPC
        ix = idxs[b]
        cnt = len(ix)

        qT = np.ascontiguousarray(q[b, :, h0:h0 + HPC, :].transpose(2, 1, 0))
        kT = np.zeros((D, HPC, Ske), np.float32)
        # fold the 1/sqrt(D) softmax scale into K (exact: 0.125 = 2^-3)
        kT[:, :, :cnt] = (k[b][ix][:, h0:h0 + HPC, :] * 0.125).transpose(2, 1, 0)
        v1_full = np.zeros((HPC, Ske, D + 1), np.float32)
        v1_full[:, :cnt, :D] = v[b][ix][:, h0:h0 + HPC, :].transpose(1, 0, 2)
        v1_full[:, :cnt, D] = 1.0
        v1 = np.ascontiguousarray(
            v1_full.reshape(HPC, T, 128, D + 1).transpose(2, 0, 1, 3)
        ).astype(ml_dtypes.bfloat16)
        in_maps.append({"qT": qT, "kT": kT, "v1": v1})

    if T not in _CACHE:
        _CACHE[T] = _build_program(T)
    nc = _CACHE[T]

    res = run_bass_kernel_spmd(nc, in_maps, core_ids=list(range(N_CORES)))

    out = np.zeros((B, Sq, H, D), np.float32)
    for c in range(N_CORES):
        b = c // CPB
        h0 = (c % CPB) * HPC
        oc = res.results[c]["o"]  # (HPC, NCHUNK, 128, 8, D)
        for i in range(HPC):
            # row order within a chunk is (i_sub, p): q = c*1024 + i_sub*128 + p
            out[b, :, h0 + i, :] = oc[i].transpose(0, 2, 1, 3).reshape(Sq, D)
    return out


# revision 13
# speedup vs baseline: 1.0227x; 1.0075x over previous
"""Cross-attention Trainium2 kernel (Bass/Tile), sharded over 8 NeuronCores.

Problem: B=2, Sq=Sk=2048, H=16, D=64, fp32, with a boolean key-padding mask
(True = keep).  softmax(q @ k^T / sqrt(D) + mask_bias) @ v per (batch, head).

Sharding: the 32 (b, h) pairs are split 4-per-core (cores 0-3 -> b=0,
cores 4-7 -> b=1).  Masked-out keys are compacted away on the host (the
reference's additive -10000 bias makes exp() underflow to exactly 0 in fp32,
so dropping those keys is exact); the kept keys are padded with zero rows up
to a multiple of 128.  Padding rows contribute exp(0)=1 "probabilities", but
their V rows AND their entry in the appended ones-column are 0, so they add
exactly nothing to either numerator or denominator.

Device math per (core, head h, 1024-query chunk c), T = Ske/128 key tiles:
  S^T[t]   = K^T[t].T @ Q^T            (PE; [128 keys, 1024 queries], c=D=64)
  P[t]     = exp(S^T[t])               (split across TWO engines:
               ~60%% of tiles: ACT exp PSUM->SBUF(bf16)
               ~40%% of tiles: DVE copy PSUM->SBUF, then GPSIMD pow(e, s)
               -- the 1/sqrt(D) scale is folded into K on the host)
  O[j]    += P[t][:,j*128:...].T @ V1[t]   (PE; V1 = [V | ones] bf16,
                                            [128 queries, 65] psum acc)
  out      = O[:, :64] * 1/O[:, 64]    (DVE reciprocal + broadcast mult)

Keeping queries on the PSUM partition dim for the PV accumulation (free dim
65 instead of 512) halves PE time vs streaming P columns, and removes the
transpose epilogue entirely.  P/V in bf16 keep the PV matmuls at 1 cyc/row.
No row-max subtraction is needed: scores are ~N(0,1), exp() cannot overflow
fp32 and matches softmax-with-max to ~1e-6.
"""
import math
import numpy as np
from contextlib import ExitStack

import ml_dtypes

import concourse.bass as bass
import concourse.tile as tile
from concourse import bacc, mybir
from concourse.bass_utils import run_bass_kernel_spmd

f32 = mybir.dt.float32
f32r = mybir.dt.float32r  # tf32-like matmul dtype: 1 cyc/row vs 4 for fp32
bf16 = mybir.dt.bfloat16

B, Sq, Sk, H, D = 2, 2048, 2048, 16, 64
N_CORES = 8
CPB = N_CORES // B          # cores per batch item (4)
HPC = H // CPB              # heads per core (4)
NCHUNK = Sq // 1024         # query chunks of 1024 per head (2)

_CACHE: dict[int, "bacc.Bacc"] = {}

# greedy channel-balancer weights: engine-busy ns per [128,1024] tile
_ACT_COST = 1038.0   # 1024/1.2GHz + psum/sbuf access
_POOL_COST = 1517.0  # q7 launch + 1024/(1.2GHz*0.6)


def _build_program(T: int) -> "bacc.Bacc":
    """Build + compile the per-core Bass program for Ske = T*128 kept keys."""
    Ske = T * 128
    nc = bacc.Bacc("TRN2", target_bir_lowering=False, debug=False)

    qT = nc.dram_tensor("qT", [D, HPC, Sq], f32r, kind="ExternalInput").ap()
    kT = nc.dram_tensor("kT", [D, HPC, Ske], f32r, kind="ExternalInput").ap()
    v1 = nc.dram_tensor("v1", [128, HPC, T, D + 1], bf16, kind="ExternalInput").ap()
    o = nc.dram_tensor("o", [HPC, NCHUNK, 128, 8, D], f32, kind="ExternalOutput").ap()

    Exp = mybir.ActivationFunctionType.Exp
    E_CONST = float(math.e)

    with tile.TileContext(nc) as tc, ExitStack() as ctx:
        const = ctx.enter_context(tc.tile_pool(name="const", bufs=1))
        ring = ctx.enter_context(tc.tile_pool(name="ring", bufs=3, space="PSUM"))
        accp = ctx.enter_context(tc.tile_pool(name="accp", bufs=2, space="PSUM"))
        ppool = ctx.enter_context(tc.tile_pool(name="ppool", bufs=18))
        scp = ctx.enter_context(tc.tile_pool(name="scp", bufs=5))
        osbp = ctx.enter_context(tc.tile_pool(name="osbp", bufs=3))
        rp = ctx.enter_context(tc.tile_pool(name="rp", bufs=2))

        # warm the ACT exp table before the main loop
        warm = const.tile([128, 1], f32, tag="warm", name="warm")
        nc.gpsimd.memset(warm[:], 0.0)
        nc.scalar.activation(warm[:], warm[:], Exp, scale=1.0)

        # e-filled tile: base operand for the GPSIMD pow(e, s) exp channel
        e_sb = const.tile([128, 1024], f32, tag="e", name="e_sb")
        nc.vector.memset(e_sb[:], E_CONST)

        q_sb = const.tile([D, HPC, Sq], f32r, tag="q", name="q_sb")
        k_sb = const.tile([D, HPC, Ske], f32r, tag="k", name="k_sb")
        v_sb = const.tile([128, HPC, T, D + 1], bf16, tag="v", name="v_sb")
        # stage the critical first tiles first; keep ALL loads off the scalar
        # queue (a DMA issue occupies the ACT sequencer for >1us and delays
        # the first exp by several us)
        nc.sync.dma_start(k_sb[:, 0, :128], kT[:, 0, :128])
        nc.scalar.dma_start(q_sb[:, 0, :1024], qT[:, 0, :1024])
        if Ske > 128:
            nc.sync.dma_start(k_sb[:, 0, 128:], kT[:, 0, 128:])
        nc.scalar.dma_start(q_sb[:, 0, 1024:], qT[:, 0, 1024:])
        for h in range(1, HPC):
            nc.sync.dma_start(k_sb[:, h], kT[:, h])
            nc.scalar.dma_start(q_sb[:, h], qT[:, h])
        nc.gpsimd.dma_start(v_sb[:], v1[:])

        vheads = [(h, c) for h in range(HPC) for c in range(NCHUNK)]
        vacc = {}            # (h, c) -> (acc0, acc1) psum tiles
        act_load = 0.0       # greedy channel balancer state
        pool_load = 0.0

        def emit_pv(h, c, t, p):
            if t == 0:
                a0 = accp.tile([128, 4, D + 1], f32, tag="acc", name="a0")
                a1 = accp.tile([128, 4, D + 1], f32, tag="acc", name="a1")
                vacc[(h, c)] = (a0, a1)
            a0, a1 = vacc[(h, c)]
            # PSUM start/stop semantics are per 2KB zero-region (bank): the
            # FIRST matmul into a bank carries start=True (clears the whole
            # bank's has_written bits); the other 3 groups' first writes then
            # overwrite their own pending-zero bytes.  One stop on the last
            # matmul into the bank.
            for j in range(8):
                acc = a0 if j < 4 else a1
                nc.tensor.matmul(
                    acc[:, j % 4, :], p[:, j * 128:(j + 1) * 128],
                    v_sb[:, h, t],
                    start=(t == 0 and j % 4 == 0),
                    stop=(t == T - 1 and j % 4 == 3),
                )
            if t == T - 1:
                emit_epilogue(h, c)

        def emit_epilogue(h, c):
            a0, a1 = vacc.pop((h, c))
            r = rp.tile([128, 8], f32, tag="r", name="r")
            nc.vector.reciprocal(r[:, 0:4], a0[:, :, D])
            nc.vector.reciprocal(r[:, 4:8], a1[:, :, D])
            ot = osbp.tile([128, 8, D], f32, tag="ot", name="ot")
            nc.vector.tensor_tensor(
                ot[:, 0:4, :], a0[:, :, :D],
                r[:, 0:4, None].to_broadcast((128, 4, D)), mybir.AluOpType.mult,
            )
            nc.vector.tensor_tensor(
                ot[:, 4:8, :], a1[:, :, :D],
                r[:, 4:8, None].to_broadcast((128, 4, D)), mybir.AluOpType.mult,
            )
            nc.sync.dma_start(o[h, c], ot[:])

        # vhead-phase software pipeline: while vhead v's scores/exps stream
        # through the ring, run the PV bursts of vhead v-1 (whose P tiles are
        # all ready, so they never stall the in-order PE queue).  PV bursts
        # are emitted BEFORE each score matmul so a ring-slot wait cannot
        # block ready PV work behind it.
        prev = None          # (h, c, [p tiles]) of the previous vhead
        for vi, (h, c) in enumerate(vheads):
            ptiles = []
            for t in range(T):
                if prev is not None:
                    emit_pv(prev[0], prev[1], t, prev[2][t])
                ps = ring.tile([128, 1024], f32, tag="s", name="ps")
                for jj in range(2):
                    q0 = c * 1024 + jj * 512
                    nc.tensor.matmul(
                        ps[:, jj * 512:(jj + 1) * 512],
                        k_sb[:, h, t * 128:(t + 1) * 128],
                        q_sb[:, h, q0:q0 + 512],
                        start=True, stop=True,
                    )
                p = ppool.tile([128, 1024], bf16, tag="p", name="p")
                if act_load + _ACT_COST <= pool_load + _POOL_COST:
                    act_load += _ACT_COST
                    nc.scalar.activation(p[:], ps[:], Exp, scale=1.0)
                else:
                    pool_load += _POOL_COST
                    s2 = scp.tile([128, 1024], f32, tag="s2", name="s2")
                    nc.vector.tensor_copy(out=s2[:], in_=ps[:])
                    nc.gpsimd.tensor_tensor(p[:], e_sb[:], s2[:],
                                            mybir.AluOpType.pow)
                ptiles.append(p)
            prev = (h, c, ptiles)
        for t in range(T):
            emit_pv(prev[0], prev[1], t, prev[2][t])

    nc.compile()
    return nc


def kernel(q, kv, key_padding_mask):
    q = np.ascontiguousarray(np.asarray(q, dtype=np.float32))
    kv = np.asarray(kv, dtype=np.float32)
    mask = np.asarray(key_padding_mask).astype(bool)
    k = kv[:, :, 0]  # (B, Sk, H, D)
    v = kv[:, :, 1]

    # Host-side compaction of masked-out keys (exact: exp(-10000) == 0 in
    # fp32).  If every key of a batch item is masked, the -10000 bias is a
    # constant and softmax ignores it -> fall back to keeping all keys.
    idxs = []
    for b in range(B):
        ix = np.nonzero(mask[b])[0]
        if len(ix) == 0:
            ix = np.arange(Sk)
        idxs.append(ix)
    T = int(np.ceil(max(len(ix) for ix in idxs) / 128))
    Ske = T * 128

    in_maps = []
    for c in range(N_CORES):
        b = c // CPB
        h0 = (c % CPB) * HPC
        ix = idxs[b]
        cnt = len(ix)

        qT = np.ascontiguousarray(q[b, :, h0:h0 + HPC, :].transpose(2, 1, 0))
        kT = np.zeros((D, HPC, Ske), np.float32)
        # fold the 1/sqrt(D) softmax scale into K (exact: 0.125 = 2^-3)
        kT[:, :, :cnt] = (k[b][ix][:, h0:h0 + HPC, :] * 0.125).transpose(2, 1, 0)
        v1_full = np.zeros((HPC, Ske, D + 1), np.float32)
        v1_full[:, :cnt, :D] = v[b][ix][:, h0:h0 + HPC, :].transpose(1, 0, 2)
        v1_full[:, :cnt, D] = 1.0
        v1 = np.ascontiguousarray(
            v1_full.reshape(HPC, T, 128, D + 1).transpose(2, 0, 1, 3)
        ).astype(ml_dtypes.bfloat16)
        in_maps.append({"qT": qT, "kT": kT, "v1": v1})

    if T not in _CACHE:
        _CACHE[T] = _build_program(T)
    nc = _CACHE[T]

    res = run_bass_kernel_spmd(nc, in_maps, core_ids=list(range(N_CORES)))

    out = np.zeros((B, Sq, H, D), np.float32)
    for c in range(N_CORES):
        b = c // CPB
        h0 = (c % CPB) * HPC
        oc = res.results[c]["o"]  # (HPC, NCHUNK, 128, 8, D)
        for i in range(HPC):
            # row order within a chunk is (i_sub, p): q = c*1024 + i_sub*128 + p
            out[b, :, h0 + i, :] = oc[i].transpose(0, 2, 1, 3).reshape(Sq, D)
    return out
